# revision 60
# baseline (speedup 1.0000x reference)
"""Trainium2 Bass kernel for nn_MultiHeadedAttention_4604204941604.

Multi-headed attention with a distance-MLP reweighting term:
  out = ((softmax(mask(QK^T/8)) * distMLP(d)^2) masked) @ V @ Wo

Structural simplifications specific to this problem instance:

1. MLP collapse: the distance-MLP biases (db1..db4) are all zero and
   src_distances >= 0, so the whole MLP collapses to dist = C * d with
   scalar C computed on the host (validity asserted) and applied
   on-device inside the dist^2 Square activation.

2. Mask compaction: rows/keys with mask==0 produce exactly-zero output
   rows / contribute nothing.  The host compacts each core's query rows
   to the valid ones (pad to NQP) and the key axis to the valid keys
   (pad to NKP), with the core's own query rows FIRST in key order so
   the score diagonal (self-attention suppression) sits at fixed
   positions (key k == query k) for every core -> single SPMD program.

3. Transposed-score topology: scores are computed directly as
   sT[k, q] with K-chunk-stationary matmuls, so the probability matrix
   is already keys-on-partitions for the AV matmul -- no PE transposes.
   - softmax denominator: den rows 0 (head even) / 32 (head odd) of one
     psum region, accumulated in a single group whose stationaries are
     vmask columns (zero-padded keys dropped for free; filler columns
     keep the unused rows finite so their reciprocal stays finite).
   - diagonal suppression: den reads e *= (1-I) (gpsimd); the AV path
     gets its diagonal zeros from the host-zeroed d^2 diagonal.
   - per-query 1/den: reciprocal straight to bf16, broadcast to
     [128, q] with an E33 stationary matmul, multiplied into the AV
     output during the PSUM->SBUF evacuation.

Sharding: core c handles batch b = c//4, query rows 256*(c%4)..+256.

Scheduling (the PE clock-gates to half speed after any ~0.7us+ idle
gap and only re-warms during long dense streams, so the whole program
is laid out to keep the tensor engine saturated):
  - inputs stream over three DMA queues (sync/scalar HWDGE ~145GB/s,
    gpsimd SWDGE ~65GB/s) ordered by need time; a zero-matmul warmup
    covers the initial DMA wait.
  - all score matmuls for three head pairs run before the V
    projection (whose inputs arrive meanwhile); reduce stages are
    split into A (den+AV+reciprocal issue) and B (broadcast+evacuate)
    with the Wo output-projection matmuls interleaved as PE filler so
    the reciprocal latency never stalls the in-order PE queue.
  - score and AV matmuls are emitted chunk-major across each head
    pair: the two heads occupy distinct PE row/column groups
    (partitions 0-63 vs 64-127), so adjacent matmuls execute
    concurrently on the 32x32 sub-arrays.  The pair's AV halves form
    ONE accumulation group (start only on the very first matmul; the
    odd head's first chunk lands on clear has_written bits and
    overwrites) because both halves live in the same PSUM bank.
  - everything shares one rotating 2-bank PSUM slot plus a 1-bank
    reduce slot; the output leaves as bf16 to halve the tail DMA.
"""

import os
import sys
import types

sys.path.insert(0, "/opt/trn_rl_repo")

import numpy as np
import ml_dtypes

import concourse.bass as bass
import concourse.bacc as bacc
import concourse.mybir as mybir
from concourse import tile

BF16 = mybir.dt.bfloat16
F32 = mybir.dt.float32
NPBF16 = ml_dtypes.bfloat16
F8 = mybir.dt.float8e4
NPF8 = ml_dtypes.float8_e4m3fn
ESC = 0.125

B, N, D, H = 2, 1024, 512, 8
DK = D // H  # 64
NCORES = 8
RPC = N * B // NCORES  # 256 query rows per core
SW = 512  # psum bank width in f32

_cache = {}


def _install_ntff_hook():
    try:
        from antenv.axon_hooks import get_axon_ntff_profile_hook  # noqa: F401
        return
    except ImportError:
        pass
    import antenv
    mod = types.ModuleType("antenv.axon_hooks")
    _hook = [None]
    mod.set_axon_ntff_profile_hook = lambda h: _hook.__setitem__(0, h)
    mod.get_axon_ntff_profile_hook = lambda: _hook[0]
    sys.modules["antenv.axon_hooks"] = mod
    antenv.axon_hooks = mod
    try:
        from trn_agent_boot.trn_boot import _ntff_profile_via_ctypes
        mod.set_axon_ntff_profile_hook(
            _ntff_profile_via_ctypes("/opt/axon/libaxon_pjrt.so"))
    except Exception:
        pass


def _build_program(NQP, NKP):
    """NQP: padded valid-query rows (mult of 32, 128 < NQP <= 256).
    NKP: padded valid-key count (mult of 64, > 512)."""
    KC = (NKP + 127) // 128            # key chunks of <=128
    KCH = [(128 * c, min(128, NKP - 128 * c)) for c in range(KC)]
    CPB = SW // NQP                    # score chunks per psum bank
    SB = (KC + CPB - 1) // CPB         # psum banks for one head's scores
    CW = CPB * NQP                     # used columns per score bank
    MQT = [(0, 128), (128, NQP - 128)]  # m-splits for the Wo stage
    assert 128 < NQP <= 256 and CPB >= 2 and SB == 2, (NQP, NKP)

    nc = bacc.Bacc("TRN2", target_bir_lowering=False, debug=False)

    # [D, X] tensors arrive pre-packed partition-major: [128, 4, X] with
    # element (p, j, x) = T[128j + p, x], contiguous per partition.
    d_qT = nc.dram_tensor("qT", (128, 4, NQP), BF16, kind="ExternalInput")
    d_kT = nc.dram_tensor("kT", (128, 4, NKP), BF16, kind="ExternalInput")
    d_vT = nc.dram_tensor("vT", (128, 4, NKP), BF16, kind="ExternalInput")
    d_wq = nc.dram_tensor("wq", (128, 4, D), BF16, kind="ExternalInput")
    d_wk = nc.dram_tensor("wk", (128, 4, D), BF16, kind="ExternalInput")
    d_wv = nc.dram_tensor("wv", (128, 4, D), BF16, kind="ExternalInput")
    d_dTp = nc.dram_tensor("dTp", (128, SB, CW), BF16, kind="ExternalInput")
    d_vm66 = nc.dram_tensor("vm66", (128, KC, 66), BF16, kind="ExternalInput")
    d_e33 = nc.dram_tensor("e33", (33, 128), BF16, kind="ExternalInput")
    d_cscale = nc.dram_tensor("cscale", (128, 1), F32, kind="ExternalInput")
    d_wo8p = nc.dram_tensor("wo8p", (128, H // 2, D), BF16,
                            kind="ExternalInput")
    d_out = nc.dram_tensor("out", (NQP, D), BF16, kind="ExternalOutput")

    with tile.TileContext(nc) as tc:
        with (
            tc.tile_pool(name="const", bufs=1) as cp,
            tc.tile_pool(name="work", bufs=2) as wp,
            tc.tile_pool(name="small", bufs=2) as sp,
        ):
            # ---- input DMAs, ordered so compute can start early ----
            qTin = cp.tile([128, 4, NQP], BF16, tag="qTin")
            kTin = cp.tile([128, 4, NKP], BF16, tag="kTin")
            vTin = cp.tile([128, 4, NKP], BF16, tag="vTin")
            wq = cp.tile([128, 4, D], BF16, tag="wq")
            wk = cp.tile([128, 4, D], BF16, tag="wk")
            wv = cp.tile([128, 4, D], BF16, tag="wv")
            # three DMA issue queues (sync + scalar HWDGE, gpsimd SWDGE),
            # each ordered by need time; large tensors split by j-slice so
            # queues share the load of each projection's inputs
            dTp = cp.tile([128, SB, CW], BF16, tag="dTp")
            vm66 = cp.tile([128, KC, 66], BF16, tag="vm66")
            E33 = cp.tile([33, 128], BF16, tag="E33")
            cscale = cp.tile([128, 1], F32, tag="cscale")
            wo8p = cp.tile([128, H // 2, D], BF16, tag="wo8p")
            # HW queues (sync/scalar ~145GB/s) carry the early-needed
            # tensors; the slower gpsimd SWDGE queue (~65GB/s) carries
            # late-needed ones plus one K/V slice each as ballast
            nc.scalar.dma_start(wq[:], d_wq[:])
            nc.gpsimd.dma_start(qTin[:], d_qT[:])
            nc.sync.dma_start(wk[:], d_wk[:])
            nc.scalar.dma_start(kTin[:, 0:2, :], d_kT[:, 0:2, :])
            nc.gpsimd.dma_start(kTin[:, 2:4, :], d_kT[:, 2:4, :])
            nc.scalar.dma_start(vTin[:, 0:2, :], d_vT[:, 0:2, :])
            nc.sync.dma_start(wv[:], d_wv[:])
            nc.gpsimd.dma_start(dTp[:], d_dTp[:])
            nc.sync.dma_start(vTin[:, 2:4, :], d_vT[:, 2:4, :])
            nc.scalar.dma_start(wo8p[:], d_wo8p[:])
            nc.sync.dma_start(vm66[:], d_vm66[:])
            nc.scalar.dma_start(E33[:], d_e33[:])
            nc.sync.dma_start(cscale[:], d_cscale[:])

            # diag-suppression mask for score chunks 0..1: 1 everywhere
            # except (k, q): k == q (chunk 0) / 128 + k == q (chunk 1)
            m01 = cp.tile([128, 2 * NQP], BF16, tag="m01")
            nc.gpsimd.memset(m01[:], 1.0)
            nc.gpsimd.affine_select(
                out=m01[:, 0:NQP], in_=m01[:, 0:NQP],
                compare_op=mybir.AluOpType.not_equal,
                fill=0.0, base=0, pattern=[[-1, NQP]], channel_multiplier=1)
            nc.gpsimd.affine_select(
                out=m01[:, NQP:2 * NQP], in_=m01[:, NQP:2 * NQP],
                compare_op=mybir.AluOpType.not_equal,
                fill=0.0, base=128, pattern=[[-1, NQP]], channel_multiplier=1)

            # projected tensors
            qT = cp.tile([128, 4, NQP], BF16, tag="qTp")
            kT = cp.tile([128, 4, NKP], BF16, tag="kTp")
            v = cp.tile([128, KC, D], BF16, tag="vp")
            d2T = cp.tile([128, SB, CW], BF16, tag="d2T")
            xoT = cp.tile([128, H // 2, NQP], BF16, tag="xoT")

            # ---- single compute scope: everything shares one rotating
            # 2-bank psum slot (tag "big") + the 1-bank reduce slot ("ax").
            # Emission order packs all full-clock-hungry matmul work as
            # early as possible: warmup, Q-proj, K-proj, scores for three
            # pairs, V-proj (its DMA arrives during the scores), then the
            # reduce stages and the Wo output projection interleaved. ----
            with (
                tc.tile_pool(name="pbig", bufs=3, space=bass.MemorySpace.PSUM) as pst,
                tc.tile_pool(name="pax", bufs=2, space=bass.MemorySpace.PSUM) as pax,
            ):
                warm = cp.tile([128, 512], BF16, tag="warm")
                nc.vector.memset(warm[:], 0.0)
                wps = pst.tile([128, SB, SW], F32, tag="big")
                for _ in range(12):
                    nc.tensor.matmul(wps[:, 0, :], warm[:, :128], warm[:],
                                     start=True, stop=True)
                wsink = cp.tile([128, 1], F32, tag="wsink")
                nc.vector.tensor_copy(wsink[:], wps[:, 0, :1])

                for i in range(4):
                    ps = pst.tile([128, SB, SW], F32, tag="big")
                    for j in range(4):
                        nc.tensor.matmul(ps[:, 0, 0:NQP],
                                         wq[:, j, 128 * i:128 * i + 128],
                                         qTin[:, j, :],
                                         start=(j == 0), stop=(j == 3))
                    if i % 2 == 0:
                        nc.scalar.copy(qT[:, i, :], ps[:, 0, 0:NQP])
                    else:
                        nc.vector.tensor_copy(qT[:, i, :], ps[:, 0, 0:NQP])

                for i in range(4):
                    ps = pst.tile([128, SB, SW], F32, tag="big")
                    for j in range(4):
                        nc.tensor.matmul(ps[:, 0, :],
                                         wk[:, j, 128 * i:128 * i + 128],
                                         kTin[:, j, 0:512],
                                         start=(j == 0), stop=(j == 3))
                    for j in range(4):
                        nc.tensor.matmul(ps[:, 1, 0:NKP - 512],
                                         wk[:, j, 128 * i:128 * i + 128],
                                         kTin[:, j, 512:NKP],
                                         start=(j == 0), stop=(j == 3))
                    if i % 2 == 0:
                        nc.scalar.copy(kT[:, i, 0:512], ps[:, 0, :])
                        nc.scalar.copy(kT[:, i, 512:NKP], ps[:, 1, 0:NKP - 512])
                    else:
                        nc.vector.tensor_copy(kT[:, i, 0:512], ps[:, 0, :])
                        nc.vector.tensor_copy(kT[:, i, 512:NKP],
                                              ps[:, 1, 0:NKP - 512])

                # d2T = (C * dT)^2 (diag zeroed on host, pads stay zero)
                nc.scalar.activation(d2T[:], dTp[:],
                                     mybir.ActivationFunctionType.Square,
                                     bias=0.0, scale=cscale[:])

                NP = H // 2
                state = [None] * NP

                def stage_vproj():
                    for c, (c0, cn) in enumerate(KCH):
                        ps = pst.tile([128, SB, SW], F32, tag="big")
                        for j in range(4):
                            nc.tensor.matmul(ps[:cn, 0, :],
                                             vTin[:, j, c0:c0 + cn],
                                             wv[:, j, :],
                                             start=(j == 0), stop=(j == 3))
                        if c % 2 == 0:
                            nc.vector.tensor_copy(v[:cn, c, :], ps[:cn, 0, :])
                        else:
                            nc.scalar.copy(v[:cn, c, :], ps[:cn, 0, :])

                def stage_scores(pr):
                    e, u = [None, None], [None, None]
                    sts = [pst.tile([128, SB, SW], F32, tag="big", name=f"st{pr}0"),
                           pst.tile([128, SB, SW], F32, tag="big", name=f"st{pr}1")]
                    # chunk-major emission: the two heads occupy distinct PE
                    # row groups (partitions 0-63 / 64-127), so adjacent
                    # matmuls execute concurrently on the array
                    for c, (c0, cn) in enumerate(KCH):
                        jb, ci = c // CPB, c % CPB
                        for hh in range(2):
                            pb = 64 * hh
                            nc.tensor.matmul(
                                sts[hh][0:cn, jb, ci * NQP:(ci + 1) * NQP],
                                kT[pb:pb + 64, pr, c0:c0 + cn],
                                qT[pb:pb + 64, pr, :],
                                start=True, stop=True)
                    for hh in range(2):
                        eh = wp.tile([128, SB, CW], BF16, tag="e", bufs=5)
                        e[hh] = eh
                        nc.scalar.activation(eh[:], sts[hh][:, :, 0:CW],
                                             mybir.ActivationFunctionType.Exp,
                                             bias=0.0, scale=ESC)
                        # u from unmasked e: d2T diagonal is host-zeroed
                        uh = wp.tile([128, SB, CW], BF16, tag="u", bufs=5)
                        u[hh] = uh
                        nc.vector.tensor_mul(uh[:], eh[:], d2T[:])
                        # suppress the diagonal for the denominator read
                        nc.gpsimd.tensor_mul(eh[:, 0, 0:2 * NQP],
                                             eh[:, 0, 0:2 * NQP], m01[:])
                    state[pr] = (e, u)

                axs = [None] * NP

                # ORDERING HAZARD -- the emission order below is
                # load-bearing.  Four experiments corrupted results:
                #   * reciprocal emitted before the AV group  -> 8e-2
                #   * den(3) hoisted before reduceB(2)        -> 6e-1
                #   * same, with a zero-stationary guard mm   -> 2e+1
                #   * wo(ff1,2) moved before reduceB(3)       -> 4e-2
                #   * five dead keep-warm mms added early
                #     (shifts every psum slot rotation phase) -> 2e-2
                # The reduce stages share psum banks in disjoint regions,
                # so Tile adds no WAR deps; correctness rests on transitive
                # engine-FIFO chains (PE order -> bcast's semaphore wait on
                # recip -> DVE order).  Do not reorder or insert psum
                # allocations here without adding explicit semaphores.
                def stage_reduceA(pr):
                    e, u = state[pr]
                    # one psum bank holds AV [:,0,:], den-bcast [:,1,:] and
                    # den rows 0/32 [0:33,2,:]; accumulation groups run
                    # strictly in sequence so has_written clears are safe
                    ax = pax.tile([128, 3, NQP], F32, tag="ax")
                    axs[pr] = ax
                    # den rows 0 / 32 in one group; filler columns keep
                    # rows 1-31 at den_h0 (finite) for the reciprocal
                    ndm = 2 * KC
                    i = 0
                    for hh in range(2):
                        for c, (c0, cn) in enumerate(KCH):
                            jb, ci = c // CPB, c % CPB
                            nc.tensor.matmul(
                                ax[0:33, 2, :],
                                vm66[0:cn, c, 33 * hh:33 * hh + 33],
                                e[hh][0:cn, jb, ci * NQP:(ci + 1) * NQP],
                                start=(i == 0), stop=(i == ndm - 1))
                            i += 1
                    # AV: ax[64*hh+i, 0, q] = sum_k v[k, i] * u[hh][k, q].
                    # One group, chunk-major across the pair: the heads sit
                    # in distinct PE column groups and run concurrently;
                    # start=True only on the very first matmul (the second
                    # head's first chunk lands on clear has_written bits,
                    # so it overwrites rather than accumulates stale data)
                    for c, (c0, cn) in enumerate(KCH):
                        jb, ci = c // CPB, c % CPB
                        for hh in range(2):
                            h = 2 * pr + hh
                            nc.tensor.matmul(
                                ax[64 * hh:64 * hh + 64, 0, :],
                                v[0:cn, c, DK * h:DK * h + DK],
                                u[hh][0:cn, jb, ci * NQP:(ci + 1) * NQP],
                                start=(c == 0 and hh == 0),
                                stop=(c == KC - 1 and hh == 1),
                                skip_group_check=True)
                    # reciprocal straight to bf16 (after the AV group: it
                    # shares the psum bank, so earlier emission would race)
                    rsb = sp.tile([33, NQP], BF16, tag="rsb")
                    with nc.allow_low_precision("softmax 1/den in bf16"):
                        nc.vector.reciprocal(rsb[:], ax[0:33, 2, :])
                    state[pr] = rsb

                def stage_reduceB(pr):
                    ax, rsb = axs[pr], state[pr]
                    # broadcast 1/den across partitions: [128, q] in psum
                    nc.tensor.matmul(ax[:, 1, :], E33[:], rsb[:],
                                     start=True, stop=True)
                    bcs = sp.tile([128, NQP], F32, tag="bcs")
                    nc.vector.tensor_copy(bcs[:], ax[:, 1, :])
                    # normalize by den while evacuating PSUM
                    nc.vector.tensor_mul(xoT[:, pr, :], ax[:, 0, :], bcs[:])
                    state[pr] = axs[pr] = None

                # Wo output accumulators live in the "big" rotation (no
                # further big allocations happen, so the open groups are
                # safe); their matmuls act as PE filler between reduce
                # stages while each pair's reciprocal drains on the DVE.
                m0, ml0 = MQT[0]
                m1, ml1 = MQT[1]

                stage_scores(0)
                stage_scores(1)
                stage_scores(2)
                stage_vproj()
                stage_reduceA(0)
                stage_scores(3)
                stage_reduceB(0)
                stage_reduceA(1)
                ff0 = pst.tile([128, SB, SW], F32, tag="big")
                ff1 = pst.tile([128, SB, SW], F32, tag="big")

                def wo(ff, mq, pr, first, last):
                    mx, mle = mq
                    nc.tensor.matmul(ff[0:mle, 0, :], xoT[:, pr, mx:mx + mle],
                                     wo8p[:, pr, :], start=first, stop=last)

                wo(ff0, MQT[0], 0, True, False)
                stage_reduceB(1)
                stage_reduceA(2)
                wo(ff0, MQT[0], 1, False, False)
                wo(ff1, MQT[1], 0, True, False)
                stage_reduceB(2)
                stage_reduceA(3)
                wo(ff0, MQT[0], 2, False, False)
                wo(ff1, MQT[1], 1, False, False)
                stage_reduceB(3)
                wo(ff0, MQT[0], 3, False, True)
                wo(ff1, MQT[1], 2, False, False)
                ob0 = wp.tile([128, D], BF16, tag="ob")
                nc.scalar.copy(ob0[0:ml0], ff0[0:ml0, 0, :])
                nc.sync.dma_start(d_out[m0:m0 + ml0, :], ob0[0:ml0])
                wo(ff1, MQT[1], 3, False, True)
                ob1 = wp.tile([128, D], BF16, tag="ob")
                nc.vector.tensor_copy(ob1[0:ml1], ff1[0:ml1, 0, :])
                nc.scalar.dma_start(d_out[m1:m1 + ml1, :], ob1[0:ml1])

    nc.compile()
    return nc


def _get_program(nqp, nkp):
    key = ("prog", nqp, nkp)
    if key not in _cache:
        _cache[key] = _build_program(nqp, nkp)
    return _cache[key]


def _pack_pm(a):
    """[512, X] -> partition-major [128, 4, X] with (p, j, x) = a[128j+p, x]."""
    return np.ascontiguousarray(a.reshape(4, 128, -1).transpose(1, 0, 2))


def kernel(**inputs):
    from concourse import bass_utils

    query = np.asarray(inputs["query"], np.float32)
    key = np.asarray(inputs["key"], np.float32)
    value = np.asarray(inputs["value"], np.float32)
    dist = np.asarray(inputs["src_distances"], np.float32)
    mask = np.asarray(inputs["mask"])
    dW1, db1 = np.asarray(inputs["dW1"], np.float64), np.asarray(inputs["db1"])
    dW2, db2 = np.asarray(inputs["dW2"], np.float64), np.asarray(inputs["db2"])
    dW3, db3 = np.asarray(inputs["dW3"], np.float64), np.asarray(inputs["db3"])
    dW4, db4 = np.asarray(inputs["dW4"], np.float64), np.asarray(inputs["db4"])

    assert all(np.all(b == 0) for b in (db1, db2, db3, db4)), \
        "distance-MLP collapse requires zero biases"
    assert dist.min() >= 0.0, "distance-MLP collapse requires d >= 0"
    u = np.maximum(dW1[0], 0.0)
    u = np.maximum(u @ dW2, 0.0)
    u = np.maximum(u @ dW3, 0.0)
    C = float(u @ dW4[:, 0])

    wq_b = _pack_pm(np.asarray(inputs["Wq"], np.float32).astype(NPBF16))
    wk_b = _pack_pm(np.asarray(inputs["Wk"], np.float32).astype(NPBF16))
    wv_b = _pack_pm(np.asarray(inputs["Wv"], np.float32).astype(NPBF16))
    wo = np.asarray(inputs["Wo"], np.float32)
    # pair-packed Wo: partition p of pair c = Wo row 128c + p
    wo8p = np.ascontiguousarray(
        wo.reshape(H // 2, 128, D).transpose(1, 0, 2)).astype(NPBF16)
    cscale = np.full((128, 1), C, np.float32)

    mf = mask != 0
    nq_max = max(int(mf[c // 4, RPC * (c % 4):RPC * (c % 4) + RPC].sum())
                 for c in range(NCORES))
    nv_max = max(int(mf[b].sum()) for b in range(B))
    NQP = max(160, 128 + ((nq_max - 128 + 31) // 32) * 32)
    NKP = max(576, 512 + ((nv_max - 512 + 63) // 64) * 64)
    KC = (NKP + 127) // 128
    CPB = SW // NQP
    SB = (KC + CPB - 1) // CPB
    CW = CPB * NQP

    e33 = np.zeros((33, 128), NPBF16)
    e33[0, 0:64] = 1.0
    e33[32, 64:128] = 1.0

    in_maps = []
    qidx_all = []
    for c in range(NCORES):
        b, r0 = c // 4, RPC * (c % 4)
        qidx = np.nonzero(mf[b, r0:r0 + RPC])[0]  # local valid query rows
        kid_own = r0 + qidx                       # global, matches q order
        other = np.nonzero(mf[b])[0]
        other = other[(other < r0) | (other >= r0 + RPC)]
        korder = np.concatenate([kid_own, other])
        nq, nv = len(qidx), len(korder)
        assert nq <= NQP and nv <= NKP, (nq, nv)
        qidx_all.append(qidx)

        qTh = np.zeros((D, NQP), NPBF16)
        qTh[:, :nq] = query[b, r0 + qidx].T.astype(NPBF16)
        kTh = np.zeros((D, NKP), NPBF16)
        kTh[:, :nv] = key[b, korder].T.astype(NPBF16)
        vTh = np.zeros((D, NKP), NPBF16)
        vTh[:, :nv] = value[b, korder].T.astype(NPBF16)
        # transposed distances in the banked score layout: chunk c at
        # [:, c//CPB, (c%CPB)*NQP : +NQP]; diagonal zeroed (suppressed
        # self-attention) so u = e * d2 needs no separate mask
        dh = dist[b, r0 + qidx][:, korder].astype(np.float32)  # [nq, nv]
        np.fill_diagonal(dh, 0.0)
        dT = np.zeros((128, SB * CW), NPBF16)
        for ck in range(KC):
            k0 = 128 * ck
            kn = min(128, nv - k0)
            if kn <= 0:
                break
            col0 = (ck // CPB) * CW + (ck % CPB) * NQP
            dT[:kn, col0:col0 + nq] = dh[:, k0:k0 + kn].T.astype(NPBF16)
        # den stationaries: h0 window cols [0..32]: col0 = vmask,
        # cols1-31 = vmask (finite filler rows), col32 = 0;
        # h1 window cols [33..65]: col65 = vmask, rest 0.
        vm66 = np.zeros((128, KC, 66), NPBF16)
        for ck in range(KC):
            kn = min(128, max(0, nv - 128 * ck))
            vm66[:kn, ck, 0:32] = 1.0
            vm66[:kn, ck, 65] = 1.0
        in_maps.append({
            "qT": _pack_pm(qTh), "kT": _pack_pm(kTh), "vT": _pack_pm(vTh),
            "dTp": dT.reshape(128, SB, CW),
            "vm66": vm66, "e33": e33, "cscale": cscale,
            "wq": wq_b, "wk": wk_b, "wv": wv_b, "wo8p": wo8p,
        })

    trace = os.environ.get("BASS_KERNEL_TRACE", "0") == "1"
    if trace:
        _install_ntff_hook()

    prog = _get_program(NQP, NKP)
    res = bass_utils.run_bass_kernel_spmd(
        prog, in_maps, core_ids=list(range(NCORES)), trace=trace)

    out = np.zeros((B, N, D), np.float32)
    for c in range(NCORES):
        b, r0 = c // 4, RPC * (c % 4)
        qidx = qidx_all[c]
        out[b, r0 + qidx] = res.results[c]["out"][:len(qidx)].astype(np.float32)
    kernel.last_exec_time_ns = res.exec_time_ns
    return out


kernel.last_exec_time_ns = None


# revision 62
# speedup vs baseline: 1.0583x; 1.0583x over previous
"""Trainium2 Bass kernel for nn_MultiHeadedAttention_4604204941604.

Multi-headed attention with a distance-MLP reweighting term:
  out = ((softmax(mask(QK^T/8)) * distMLP(d)^2) masked) @ V @ Wo

Structural simplifications specific to this problem instance:

1. MLP collapse: the distance-MLP biases (db1..db4) are all zero and
   src_distances >= 0, so the whole MLP collapses to dist = C * d with
   scalar C computed on the host (validity asserted) and applied
   on-device inside the dist^2 Square activation.

2. Mask compaction: rows/keys with mask==0 produce exactly-zero output
   rows / contribute nothing.  The host compacts each core's query rows
   to the valid ones (pad to NQP) and the key axis to the valid keys
   (pad to NKP), with the core's own query rows FIRST in key order so
   the score diagonal (self-attention suppression) sits at fixed
   positions (key k == query k) for every core -> single SPMD program.

3. Transposed-score topology: scores are computed directly as
   sT[k, q] with K-chunk-stationary matmuls, so the probability matrix
   is already keys-on-partitions for the AV matmul -- no PE transposes.
   - softmax denominator: den rows 0 (head even) / 32 (head odd) of one
     psum region, accumulated in a single group whose stationaries are
     vmask columns (zero-padded keys dropped for free; filler columns
     keep the unused rows finite so their reciprocal stays finite).
   - diagonal suppression: den reads e *= (1-I) (gpsimd); the AV path
     gets its diagonal zeros from the host-zeroed d^2 diagonal.
   - per-query 1/den: reciprocal straight to bf16, broadcast to
     [128, q] with an E33 stationary matmul, multiplied into the AV
     output during the PSUM->SBUF evacuation.

Sharding: core c handles batch b = c//4, query rows 256*(c%4)..+256.

Scheduling (the PE clock-gates to half speed after any ~0.7us+ idle
gap and only re-warms during long dense streams, so the whole program
is laid out to keep the tensor engine saturated):
  - inputs stream over three DMA queues (sync/scalar HWDGE ~145GB/s,
    gpsimd SWDGE ~65GB/s) ordered by need time; a zero-matmul warmup
    covers the initial DMA wait.
  - all score matmuls for three head pairs run before the V
    projection (whose inputs arrive meanwhile); reduce stages are
    split into A (den+AV+reciprocal issue) and B (broadcast+evacuate)
    with the Wo output-projection matmuls interleaved as PE filler so
    the reciprocal latency never stalls the in-order PE queue.
  - score and AV matmuls are emitted chunk-major across each head
    pair: the two heads occupy distinct PE row/column groups
    (partitions 0-63 vs 64-127), so adjacent matmuls execute
    concurrently on the 32x32 sub-arrays.  The pair's AV halves form
    ONE accumulation group (start only on the very first matmul; the
    odd head's first chunk lands on clear has_written bits and
    overwrites) because both halves live in the same PSUM bank.
  - everything shares one rotating 2-bank PSUM slot plus a 1-bank
    reduce slot; the output leaves as bf16 to halve the tail DMA.
"""

import os
import sys
import types

sys.path.insert(0, "/opt/trn_rl_repo")

import numpy as np
import ml_dtypes

import concourse.bass as bass
import concourse.bacc as bacc
import concourse.mybir as mybir
from concourse import tile

BF16 = mybir.dt.bfloat16
F32 = mybir.dt.float32
NPBF16 = ml_dtypes.bfloat16
F8 = mybir.dt.float8e4
NPF8 = ml_dtypes.float8_e4m3fn
ESC = 0.125

B, N, D, H = 2, 1024, 512, 8
DK = D // H  # 64
NCORES = 8
RPC = N * B // NCORES  # 256 query rows per core
SW = 512  # psum bank width in f32

_cache = {}


def _install_ntff_hook():
    try:
        from antenv.axon_hooks import get_axon_ntff_profile_hook  # noqa: F401
        return
    except ImportError:
        pass
    import antenv
    mod = types.ModuleType("antenv.axon_hooks")
    _hook = [None]
    mod.set_axon_ntff_profile_hook = lambda h: _hook.__setitem__(0, h)
    mod.get_axon_ntff_profile_hook = lambda: _hook[0]
    sys.modules["antenv.axon_hooks"] = mod
    antenv.axon_hooks = mod
    try:
        from trn_agent_boot.trn_boot import _ntff_profile_via_ctypes
        mod.set_axon_ntff_profile_hook(
            _ntff_profile_via_ctypes("/opt/axon/libaxon_pjrt.so"))
    except Exception:
        pass


def _build_program(NQP, NKP):
    """NQP: padded valid-query rows (mult of 32, 128 < NQP <= 256).
    NKP: padded valid-key count (mult of 64, > 512)."""
    KC = (NKP + 127) // 128            # key chunks of <=128
    KCH = [(128 * c, min(128, NKP - 128 * c)) for c in range(KC)]
    CPB = SW // NQP                    # score chunks per psum bank
    SB = (KC + CPB - 1) // CPB         # psum banks for one head's scores
    CW = CPB * NQP                     # used columns per score bank
    MQT = [(0, 128), (128, NQP - 128)]  # m-splits for the Wo stage
    assert 128 < NQP <= 256 and CPB >= 2 and SB == 2, (NQP, NKP)

    nc = bacc.Bacc("TRN2", target_bir_lowering=False, debug=False)

    # [D, X] tensors arrive pre-packed partition-major: [128, 4, X] with
    # element (p, j, x) = T[128j + p, x], contiguous per partition.
    d_qT = nc.dram_tensor("qT", (128, 4, NQP), BF16, kind="ExternalInput")
    d_kT = nc.dram_tensor("kT", (128, 4, NKP), BF16, kind="ExternalInput")
    d_vT = nc.dram_tensor("vT", (128, 4, NKP), BF16, kind="ExternalInput")
    d_wq = nc.dram_tensor("wq", (128, 4, D), BF16, kind="ExternalInput")
    d_wk = nc.dram_tensor("wk", (128, 4, D), BF16, kind="ExternalInput")
    d_wv = nc.dram_tensor("wv", (128, 4, D), BF16, kind="ExternalInput")
    d_dTp = nc.dram_tensor("dTp", (128, SB, CW), BF16, kind="ExternalInput")
    d_vm66 = nc.dram_tensor("vm66", (128, KC, 66), BF16, kind="ExternalInput")
    d_e33 = nc.dram_tensor("e33", (33, 128), BF16, kind="ExternalInput")
    d_cscale = nc.dram_tensor("cscale", (128, 1), F32, kind="ExternalInput")
    d_wo8p = nc.dram_tensor("wo8p", (128, H // 2, D), BF16,
                            kind="ExternalInput")
    d_out = nc.dram_tensor("out", (NQP, D), BF16, kind="ExternalOutput")

    with tile.TileContext(nc) as tc:
        with (
            tc.tile_pool(name="const", bufs=1) as cp,
            tc.tile_pool(name="work", bufs=2) as wp,
            tc.tile_pool(name="small", bufs=2) as sp,
        ):
            # ---- input DMAs, ordered so compute can start early ----
            qTin = cp.tile([128, 4, NQP], BF16, tag="qTin")
            kTin = cp.tile([128, 4, NKP], BF16, tag="kTin")
            vTin = cp.tile([128, 4, NKP], BF16, tag="vTin")
            wq = cp.tile([128, 4, D], BF16, tag="wq")
            wk = cp.tile([128, 4, D], BF16, tag="wk")
            wv = cp.tile([128, 4, D], BF16, tag="wv")
            # three DMA issue queues (sync + scalar HWDGE, gpsimd SWDGE),
            # each ordered by need time; large tensors split by j-slice so
            # queues share the load of each projection's inputs
            dTp = cp.tile([128, SB, CW], BF16, tag="dTp")
            vm66 = cp.tile([128, KC, 66], BF16, tag="vm66")
            E33 = cp.tile([33, 128], BF16, tag="E33")
            cscale = cp.tile([128, 1], F32, tag="cscale")
            wo8p = cp.tile([128, H // 2, D], BF16, tag="wo8p")
            # HW queues (sync/scalar ~145GB/s) carry the early-needed
            # tensors; the slower gpsimd SWDGE queue (~65GB/s) carries
            # late-needed ones plus one K/V slice each as ballast
            nc.scalar.dma_start(wq[:], d_wq[:])
            nc.gpsimd.dma_start(qTin[:], d_qT[:])
            nc.sync.dma_start(wk[:], d_wk[:])
            nc.scalar.dma_start(kTin[:, 0:2, :], d_kT[:, 0:2, :])
            nc.gpsimd.dma_start(kTin[:, 2:4, :], d_kT[:, 2:4, :])
            nc.scalar.dma_start(vTin[:, 0:2, :], d_vT[:, 0:2, :])
            nc.sync.dma_start(wv[:], d_wv[:])
            nc.gpsimd.dma_start(dTp[:], d_dTp[:])
            nc.sync.dma_start(vTin[:, 2:4, :], d_vT[:, 2:4, :])
            nc.scalar.dma_start(wo8p[:], d_wo8p[:])
            nc.sync.dma_start(vm66[:], d_vm66[:])
            nc.scalar.dma_start(E33[:], d_e33[:])
            nc.sync.dma_start(cscale[:], d_cscale[:])

            # diag-suppression mask for score chunks 0..1: 1 everywhere
            # except (k, q): k == q (chunk 0) / 128 + k == q (chunk 1)
            m01 = cp.tile([128, 2 * NQP], BF16, tag="m01")
            nc.gpsimd.memset(m01[:], 1.0)
            nc.gpsimd.affine_select(
                out=m01[:, 0:NQP], in_=m01[:, 0:NQP],
                compare_op=mybir.AluOpType.not_equal,
                fill=0.0, base=0, pattern=[[-1, NQP]], channel_multiplier=1)
            nc.gpsimd.affine_select(
                out=m01[:, NQP:2 * NQP], in_=m01[:, NQP:2 * NQP],
                compare_op=mybir.AluOpType.not_equal,
                fill=0.0, base=128, pattern=[[-1, NQP]], channel_multiplier=1)

            # projected tensors
            qT = cp.tile([128, 4, NQP], BF16, tag="qTp")
            kT = cp.tile([128, 4, NKP], BF16, tag="kTp")
            v = cp.tile([128, KC, D], BF16, tag="vp")
            d2T = cp.tile([128, SB, CW], BF16, tag="d2T")
            xoT = cp.tile([128, H // 2, NQP], BF16, tag="xoT")

            # ---- single compute scope: everything shares one rotating
            # 2-bank psum slot (tag "big") + the 1-bank reduce slot ("ax").
            # Emission order packs all full-clock-hungry matmul work as
            # early as possible: warmup, Q-proj, K-proj, scores for three
            # pairs, V-proj (its DMA arrives during the scores), then the
            # reduce stages and the Wo output projection interleaved. ----
            with (
                tc.tile_pool(name="pbig", bufs=3, space=bass.MemorySpace.PSUM) as pst,
                tc.tile_pool(name="pax", bufs=2, space=bass.MemorySpace.PSUM) as pax,
            ):
                warm = cp.tile([128, 512], BF16, tag="warm")
                nc.vector.memset(warm[:], 0.0)
                wps = pst.tile([128, SB, SW], F32, tag="big")
                for _ in range(12):
                    nc.tensor.matmul(wps[:, 0, :], warm[:, :128], warm[:],
                                     start=True, stop=True)
                wsink = cp.tile([128, 1], F32, tag="wsink")
                nc.vector.tensor_copy(wsink[:], wps[:, 0, :1])

                for i in range(4):
                    ps = pst.tile([128, SB, SW], F32, tag="big")
                    for j in range(4):
                        nc.tensor.matmul(ps[:, 0, 0:NQP],
                                         wq[:, j, 128 * i:128 * i + 128],
                                         qTin[:, j, :],
                                         start=(j == 0), stop=(j == 3))
                    if i % 2 == 0:
                        nc.scalar.copy(qT[:, i, :], ps[:, 0, 0:NQP])
                    else:
                        nc.vector.tensor_copy(qT[:, i, :], ps[:, 0, 0:NQP])

                for i in range(4):
                    ps = pst.tile([128, SB, SW], F32, tag="big")
                    for j in range(4):
                        nc.tensor.matmul(ps[:, 0, :],
                                         wk[:, j, 128 * i:128 * i + 128],
                                         kTin[:, j, 0:512],
                                         start=(j == 0), stop=(j == 3))
                    for j in range(4):
                        nc.tensor.matmul(ps[:, 1, 0:NKP - 512],
                                         wk[:, j, 128 * i:128 * i + 128],
                                         kTin[:, j, 512:NKP],
                                         start=(j == 0), stop=(j == 3))
                    if i % 2 == 0:
                        nc.scalar.copy(kT[:, i, 0:512], ps[:, 0, :])
                        nc.scalar.copy(kT[:, i, 512:NKP], ps[:, 1, 0:NKP - 512])
                    else:
                        nc.vector.tensor_copy(kT[:, i, 0:512], ps[:, 0, :])
                        nc.vector.tensor_copy(kT[:, i, 512:NKP],
                                              ps[:, 1, 0:NKP - 512])

                # d2T = (C * dT)^2 (diag zeroed on host, pads stay zero)
                nc.scalar.activation(d2T[:], dTp[:],
                                     mybir.ActivationFunctionType.Square,
                                     bias=0.0, scale=cscale[:])

                NP = H // 2
                state = [None] * NP

                def stage_vproj():
                    for c, (c0, cn) in enumerate(KCH):
                        ps = pst.tile([128, SB, SW], F32, tag="big")
                        for j in range(4):
                            nc.tensor.matmul(ps[:cn, 0, :],
                                             vTin[:, j, c0:c0 + cn],
                                             wv[:, j, :],
                                             start=(j == 0), stop=(j == 3))
                        if c % 2 == 0:
                            nc.vector.tensor_copy(v[:cn, c, :], ps[:cn, 0, :])
                        else:
                            nc.scalar.copy(v[:cn, c, :], ps[:cn, 0, :])

                def stage_scores(pr):
                    e, u = [None, None], [None, None]
                    sts = [pst.tile([128, SB, SW], F32, tag="big", name=f"st{pr}0"),
                           pst.tile([128, SB, SW], F32, tag="big", name=f"st{pr}1")]
                    # chunk-major emission: the two heads occupy distinct PE
                    # row groups (partitions 0-63 / 64-127), so adjacent
                    # matmuls execute concurrently on the array
                    for c, (c0, cn) in enumerate(KCH):
                        jb, ci = c // CPB, c % CPB
                        for hh in range(2):
                            pb = 64 * hh
                            nc.tensor.matmul(
                                sts[hh][0:cn, jb, ci * NQP:(ci + 1) * NQP],
                                kT[pb:pb + 64, pr, c0:c0 + cn],
                                qT[pb:pb + 64, pr, :],
                                start=True, stop=True)
                    for hh in range(2):
                        eh = wp.tile([128, SB, CW], BF16, tag="e", bufs=5)
                        e[hh] = eh
                        nc.scalar.activation(eh[:], sts[hh][:, :, 0:CW],
                                             mybir.ActivationFunctionType.Exp,
                                             bias=0.0, scale=ESC)
                        # u from unmasked e: d2T diagonal is host-zeroed
                        uh = wp.tile([128, SB, CW], BF16, tag="u", bufs=5)
                        u[hh] = uh
                        nc.vector.tensor_mul(uh[:], eh[:], d2T[:])
                        # suppress the diagonal for the denominator read
                        nc.gpsimd.tensor_mul(eh[:, 0, 0:2 * NQP],
                                             eh[:, 0, 0:2 * NQP], m01[:])
                    state[pr] = (e, u)

                axs = [None] * NP

                # ORDERING HAZARD: reduceA(k+1)'s den matmuls write the ax
                # psum slot last read by reduceB(k-1) (bufs=2 rotation) in
                # DISJOINT regions, so Tile adds no WAR dependency.  They
                # are race-free only because reduceB(k)'s bcast matmul sits
                # between them on the PE queue and waits on recip(k), which
                # follows reduceB(k-1)'s reads in the DVE FIFO.  Emitting
                # den(k+1) any earlier (e.g. before reduceB(k)) breaks that
                # transitive chain and corrupts results (measured 6e-1).
                def stage_reduceA(pr):
                    e, u = state[pr]
                    # one psum bank holds AV [:,0,:], den-bcast [:,1,:] and
                    # den rows 0/32 [0:33,2,:]; accumulation groups run
                    # strictly in sequence so has_written clears are safe
                    ax = pax.tile([128, 3, NQP], F32, tag="ax")
                    axs[pr] = ax
                    # den rows 0 / 32 in one group; filler columns keep
                    # rows 1-31 at den_h0 (finite) for the reciprocal
                    ndm = 2 * KC
                    i = 0
                    for hh in range(2):
                        for c, (c0, cn) in enumerate(KCH):
                            jb, ci = c // CPB, c % CPB
                            nc.tensor.matmul(
                                ax[0:33, 2, :],
                                vm66[0:cn, c, 33 * hh:33 * hh + 33],
                                e[hh][0:cn, jb, ci * NQP:(ci + 1) * NQP],
                                start=(i == 0), stop=(i == ndm - 1))
                            i += 1
                    # AV: ax[64*hh+i, 0, q] = sum_k v[k, i] * u[hh][k, q].
                    # One group, chunk-major across the pair: the heads sit
                    # in distinct PE column groups and run concurrently;
                    # start=True only on the very first matmul (the second
                    # head's first chunk lands on clear has_written bits,
                    # so it overwrites rather than accumulates stale data)
                    for c, (c0, cn) in enumerate(KCH):
                        jb, ci = c // CPB, c % CPB
                        for hh in range(2):
                            h = 2 * pr + hh
                            nc.tensor.matmul(
                                ax[64 * hh:64 * hh + 64, 0, :],
                                v[0:cn, c, DK * h:DK * h + DK],
                                u[hh][0:cn, jb, ci * NQP:(ci + 1) * NQP],
                                start=(c == 0 and hh == 0),
                                stop=(c == KC - 1 and hh == 1),
                                skip_group_check=True)
                    # reciprocal straight to bf16 (after the AV group: it
                    # shares the psum bank, so earlier emission would race)
                    rsb = sp.tile([33, NQP], BF16, tag="rsb")
                    with nc.allow_low_precision("softmax 1/den in bf16"):
                        nc.vector.reciprocal(rsb[:], ax[0:33, 2, :])
                    state[pr] = rsb

                def stage_reduceB(pr):
                    ax, rsb = axs[pr], state[pr]
                    # broadcast 1/den across partitions: [128, q] in psum
                    nc.tensor.matmul(ax[:, 1, :], E33[:], rsb[:],
                                     start=True, stop=True)
                    bcs = sp.tile([128, NQP], F32, tag="bcs")
                    nc.vector.tensor_copy(bcs[:], ax[:, 1, :])
                    # normalize by den while evacuating PSUM
                    nc.vector.tensor_mul(xoT[:, pr, :], ax[:, 0, :], bcs[:])
                    state[pr] = axs[pr] = None

                # Wo output accumulators live in the "big" rotation (no
                # further big allocations happen, so the open groups are
                # safe); their matmuls act as PE filler between reduce
                # stages while each pair's reciprocal drains on the DVE.
                m0, ml0 = MQT[0]
                m1, ml1 = MQT[1]

                stage_scores(0)
                stage_scores(1)
                stage_scores(2)
                stage_vproj()
                stage_reduceA(0)
                stage_scores(3)
                stage_reduceB(0)
                stage_reduceA(1)
                ff0 = pst.tile([128, SB, SW], F32, tag="big")
                ff1 = pst.tile([128, SB, SW], F32, tag="big")

                def wo(ff, mq, pr, first, last):
                    mx, mle = mq
                    nc.tensor.matmul(ff[0:mle, 0, :], xoT[:, pr, mx:mx + mle],
                                     wo8p[:, pr, :], start=first, stop=last)

                wo(ff0, MQT[0], 0, True, False)
                stage_reduceB(1)
                stage_reduceA(2)
                wo(ff0, MQT[0], 1, False, False)
                wo(ff1, MQT[1], 0, True, False)
                stage_reduceB(2)
                stage_reduceA(3)
                wo(ff0, MQT[0], 2, False, False)
                wo(ff1, MQT[1], 1, False, False)
                stage_reduceB(3)
                wo(ff0, MQT[0], 3, False, True)
                wo(ff1, MQT[1], 2, False, False)
                ob0 = wp.tile([128, D], BF16, tag="ob")
                nc.scalar.copy(ob0[0:ml0], ff0[0:ml0, 0, :])
                nc.sync.dma_start(d_out[m0:m0 + ml0, :], ob0[0:ml0])
                wo(ff1, MQT[1], 3, False, True)
                ob1 = wp.tile([128, D], BF16, tag="ob")
                nc.vector.tensor_copy(ob1[0:ml1], ff1[0:ml1, 0, :])
                nc.scalar.dma_start(d_out[m1:m1 + ml1, :], ob1[0:ml1])

    nc.compile()
    return nc


def _get_program(nqp, nkp):
    key = ("prog", nqp, nkp)
    if key not in _cache:
        _cache[key] = _build_program(nqp, nkp)
    return _cache[key]


def _pack_pm(a):
    """[512, X] -> partition-major [128, 4, X] with (p, j, x) = a[128j+p, x]."""
    return np.ascontiguousarray(a.reshape(4, 128, -1).transpose(1, 0, 2))


def kernel(**inputs):
    from concourse import bass_utils

    query = np.asarray(inputs["query"], np.float32)
    key = np.asarray(inputs["key"], np.float32)
    value = np.asarray(inputs["value"], np.float32)
    dist = np.asarray(inputs["src_distances"], np.float32)
    mask = np.asarray(inputs["mask"])
    dW1, db1 = np.asarray(inputs["dW1"], np.float64), np.asarray(inputs["db1"])
    dW2, db2 = np.asarray(inputs["dW2"], np.float64), np.asarray(inputs["db2"])
    dW3, db3 = np.asarray(inputs["dW3"], np.float64), np.asarray(inputs["db3"])
    dW4, db4 = np.asarray(inputs["dW4"], np.float64), np.asarray(inputs["db4"])

    assert all(np.all(b == 0) for b in (db1, db2, db3, db4)), \
        "distance-MLP collapse requires zero biases"
    assert dist.min() >= 0.0, "distance-MLP collapse requires d >= 0"
    u = np.maximum(dW1[0], 0.0)
    u = np.maximum(u @ dW2, 0.0)
    u = np.maximum(u @ dW3, 0.0)
    C = float(u @ dW4[:, 0])

    wq_b = _pack_pm(np.asarray(inputs["Wq"], np.float32).astype(NPBF16))
    wk_b = _pack_pm(np.asarray(inputs["Wk"], np.float32).astype(NPBF16))
    wv_b = _pack_pm(np.asarray(inputs["Wv"], np.float32).astype(NPBF16))
    wo = np.asarray(inputs["Wo"], np.float32)
    # pair-packed Wo: partition p of pair c = Wo row 128c + p
    wo8p = np.ascontiguousarray(
        wo.reshape(H // 2, 128, D).transpose(1, 0, 2)).astype(NPBF16)
    cscale = np.full((128, 1), C, np.float32)

    mf = mask != 0
    nq_max = max(int(mf[c // 4, RPC * (c % 4):RPC * (c % 4) + RPC].sum())
                 for c in range(NCORES))
    nv_max = max(int(mf[b].sum()) for b in range(B))
    NQP = max(160, 128 + ((nq_max - 128 + 31) // 32) * 32)
    NKP = max(576, 512 + ((nv_max - 512 + 63) // 64) * 64)
    KC = (NKP + 127) // 128
    CPB = SW // NQP
    SB = (KC + CPB - 1) // CPB
    CW = CPB * NQP

    e33 = np.zeros((33, 128), NPBF16)
    e33[0, 0:64] = 1.0
    e33[32, 64:128] = 1.0

    in_maps = []
    qidx_all = []
    for c in range(NCORES):
        b, r0 = c // 4, RPC * (c % 4)
        qidx = np.nonzero(mf[b, r0:r0 + RPC])[0]  # local valid query rows
        kid_own = r0 + qidx                       # global, matches q order
        other = np.nonzero(mf[b])[0]
        other = other[(other < r0) | (other >= r0 + RPC)]
        korder = np.concatenate([kid_own, other])
        nq, nv = len(qidx), len(korder)
        assert nq <= NQP and nv <= NKP, (nq, nv)
        qidx_all.append(qidx)

        qTh = np.zeros((D, NQP), NPBF16)
        qTh[:, :nq] = query[b, r0 + qidx].T.astype(NPBF16)
        kTh = np.zeros((D, NKP), NPBF16)
        kTh[:, :nv] = key[b, korder].T.astype(NPBF16)
        vTh = np.zeros((D, NKP), NPBF16)
        vTh[:, :nv] = value[b, korder].T.astype(NPBF16)
        # transposed distances in the banked score layout: chunk c at
        # [:, c//CPB, (c%CPB)*NQP : +NQP]; diagonal zeroed (suppressed
        # self-attention) so u = e * d2 needs no separate mask
        dh = dist[b, r0 + qidx][:, korder].astype(np.float32)  # [nq, nv]
        np.fill_diagonal(dh, 0.0)
        dT = np.zeros((128, SB * CW), NPBF16)
        for ck in range(KC):
            k0 = 128 * ck
            kn = min(128, nv - k0)
            if kn <= 0:
                break
            col0 = (ck // CPB) * CW + (ck % CPB) * NQP
            dT[:kn, col0:col0 + nq] = dh[:, k0:k0 + kn].T.astype(NPBF16)
        # den stationaries: h0 window cols [0..32]: col0 = vmask,
        # cols1-31 = vmask (finite filler rows), col32 = 0;
        # h1 window cols [33..65]: col65 = vmask, rest 0.
        vm66 = np.zeros((128, KC, 66), NPBF16)
        for ck in range(KC):
            kn = min(128, max(0, nv - 128 * ck))
            vm66[:kn, ck, 0:32] = 1.0
            vm66[:kn, ck, 65] = 1.0
        in_maps.append({
            "qT": _pack_pm(qTh), "kT": _pack_pm(kTh), "vT": _pack_pm(vTh),
            "dTp": dT.reshape(128, SB, CW),
            "vm66": vm66, "e33": e33, "cscale": cscale,
            "wq": wq_b, "wk": wk_b, "wv": wv_b, "wo8p": wo8p,
        })

    trace = os.environ.get("BASS_KERNEL_TRACE", "0") == "1"
    if trace:
        _install_ntff_hook()

    prog = _get_program(NQP, NKP)
    res = bass_utils.run_bass_kernel_spmd(
        prog, in_maps, core_ids=list(range(NCORES)), trace=trace)

    out = np.zeros((B, N, D), np.float32)
    for c in range(NCORES):
        b, r0 = c // 4, RPC * (c % 4)
        qidx = qidx_all[c]
        out[b, r0 + qidx] = res.results[c]["out"][:len(qidx)].astype(np.float32)
    kernel.last_exec_time_ns = res.exec_time_ns
    return out


kernel.last_exec_time_ns = None
